# revision 6
# baseline (speedup 1.0000x reference)
"""Trainium2 Bass kernel for nn_Actor_Critic (dense MLP + LSTM cell + attention).

Strategy: pure data parallel over 8 NeuronCores. Each core processes
B/8 = 4096 rows with replicated (tiny) weights.

Layout: activations are feature-major on chip ([feature_part, row_free]),
so every layer's matmul is out = W_T.T @ x_T with the contraction on the
partition dim, biases become per-partition scalars fused into the ACT
eviction, and the output stays feature-major for the next layer.
Host transposes the [B,256] inputs once and transposes outputs back.

Compute dtype: float32r (TF32-like, full 1 cycle/row PE rate, ~1.6e-4 rel
err measured on HW). Producers round on their output write (ACT/DVE), so
the f32r chain costs no extra instructions.

Embedding folding: instruction_idx has only 4 values, so
  gated_att = sigmoid(emb[idx] @ w_ta.T + b_ta) = table4[:, idx]
and the gated_att @ w_ih[:,256:512].T term of the LSTM gates is
  onehot.T @ (table4_sig @ w2.T)  -- a [4,1024] table; both tables are
built on-device in a tiny preamble. This removes a K=256 matmul per layer
chain (~20% of total PE work).
"""

import numpy as np

B = 32768
NCORES = 8
R = B // NCORES          # rows per core = 4096
NB = 512                 # rows per moving block (fp32 moving max / PSUM bank)
NBLK = R // NB           # 8 blocks per core

_CACHED = {}


def _fix_excess_waits(nc, mybir, keep=1, per_nop=1):
    """This container's walrus accepts only a limited number of sync-wait
    commands per instruction, but Tile can emit more (observed on the tail
    drain and on matmuls). Hoist excess waits onto carrier NoOps inserted
    just before the over-limit instruction on the same engine, preserving
    per-engine program order."""
    n_fixed = 0
    for f in nc.m.functions:
        for bb in f.blocks:
            insts = list(bb.instructions)
            out = []
            changed = False
            for inst in insts:
                si = inst.sync_info
                waits = list(si.on_wait) if si is not None and si.on_wait else []
                if len(waits) > keep:
                    excess, rest = waits[:-keep], waits[-keep:]
                    for i in range(0, len(excess), per_nop):
                        nop = mybir.InstNoOp(
                            name=nc.get_next_instruction_name(), ins=[], outs=[]
                        )
                        nop.engine = inst.engine
                        nop.sync_info = mybir.SyncInfo(
                            on_wait=excess[i : i + per_nop], on_update=[]
                        )
                        out.append(nop)
                    si.on_wait = rest
                    changed = True
                    n_fixed += 1
                out.append(inst)
            if changed:
                bb.instructions = out
    return n_fixed


def _build():
    import concourse.bass as bass
    import concourse.mybir as mybir
    from concourse.tile import TileContext

    f32 = mybir.dt.float32
    f32r = mybir.dt.float32r
    AF = mybir.ActivationFunctionType
    ALU = mybir.AluOpType

    nc = bass.Bass("TRN2")

    # ---- DRAM parameters (per-core shard views; host pre-transposed) ----
    d_img = nc.declare_dram_parameter("imgT", [256, R], f32, isOutput=False)
    d_hx = nc.declare_dram_parameter("hxT", [256, R], f32, isOutput=False)
    d_cx = nc.declare_dram_parameter("cxT", [256, R], f32, isOutput=False)
    d_q = nc.declare_dram_parameter("queryT", [256, R], f32, isOutput=False)
    d_oh = nc.declare_dram_parameter("ohT", [4, R], f32, isOutput=False)

    d_ta = nc.declare_dram_parameter("ta_ext", [26, 256], f32, isOutput=False)
    d_emb = nc.declare_dram_parameter("emb_ext", [26, 4], f32, isOutput=False)
    d_w1 = nc.declare_dram_parameter("w1T", [256, 1024], f32, isOutput=False)
    d_w2 = nc.declare_dram_parameter("w2T", [256, 1024], f32, isOutput=False)
    d_whh = nc.declare_dram_parameter("whhT", [256, 1024], f32, isOutput=False)
    d_bl = nc.declare_dram_parameter("b_lstm", [1024, 1], f32, isOutput=False)
    d_wba = nc.declare_dram_parameter("wbaT", [512, 512], f32, isOutput=False)
    d_bba = nc.declare_dram_parameter("b_ba", [512, 1], f32, isOutput=False)
    d_wq = nc.declare_dram_parameter("wqT", [256, 256], f32, isOutput=False)
    d_bq = nc.declare_dram_parameter("b_q", [256, 1], f32, isOutput=False)
    d_wk = nc.declare_dram_parameter("wkT", [256, 256], f32, isOutput=False)
    d_bk = nc.declare_dram_parameter("b_k", [256, 1], f32, isOutput=False)
    d_wat = nc.declare_dram_parameter("watT", [512, 256], f32, isOutput=False)
    d_bat = nc.declare_dram_parameter("b_at", [256, 1], f32, isOutput=False)
    d_wp1 = nc.declare_dram_parameter("wp1T", [256, 128], f32, isOutput=False)
    d_bp1 = nc.declare_dram_parameter("b_p1", [128, 1], f32, isOutput=False)
    d_wv1 = nc.declare_dram_parameter("wv1T", [256, 64], f32, isOutput=False)
    d_bv1 = nc.declare_dram_parameter("b_v1", [64, 1], f32, isOutput=False)
    d_wp2 = nc.declare_dram_parameter("wp2T", [128, 64], f32, isOutput=False)
    d_bp2 = nc.declare_dram_parameter("b_p2", [64, 1], f32, isOutput=False)
    d_wv2 = nc.declare_dram_parameter("wv2T", [64, 32], f32, isOutput=False)
    d_bv2 = nc.declare_dram_parameter("b_v2", [32, 1], f32, isOutput=False)
    d_wpv = nc.declare_dram_parameter("wpvT", [96, 4], f32, isOutput=False)
    d_bpv = nc.declare_dram_parameter("b_pv", [4, 1], f32, isOutput=False)

    d_hxo = nc.declare_dram_parameter("hxT_out", [256, R], f32, isOutput=True)
    d_cxo = nc.declare_dram_parameter("cxT_out", [256, R], f32, isOutput=True)
    d_pvo = nc.declare_dram_parameter("pvT_out", [4, R], f32, isOutput=True)

    # feature-major DRAM views split into 128-partition k-tiles
    imgR = d_img[:, :].rearrange("(k p) r -> p k r", p=128)
    hxR = d_hx[:, :].rearrange("(k p) r -> p k r", p=128)
    cxR = d_cx[:, :].rearrange("(k p) r -> p k r", p=128)
    qR = d_q[:, :].rearrange("(k p) r -> p k r", p=128)
    ohR = d_oh[:, :]
    hxoR = d_hxo[:, :].rearrange("(k p) r -> p k r", p=128)
    cxoR = d_cxo[:, :].rearrange("(k p) r -> p k r", p=128)
    pvoR = d_pvo[:, :]

    with TileContext(nc) as tc:
        with (
            tc.tile_pool(name="const", bufs=1) as cp,
            tc.tile_pool(name="stage", bufs=1) as sp,
            tc.tile_pool(name="inp", bufs=2) as ip,
            tc.tile_pool(name="act", bufs=1) as ap_,
            tc.tile_pool(name="act2", bufs=2) as ap2,
            tc.tile_pool(name="tmp", bufs=6) as tp,
            tc.tile_pool(name="ps", bufs=8, space="PSUM") as pp,
        ):
            # ---------- weights: DMA + round to f32r ----------
            def load_round(dram, shape3, tag):
                """DMA [K, M] weight as [128, kt, M] tiles and round to f32r."""
                kt = shape3[1]
                st = sp.tile([128, kt, shape3[2]], f32, tag="stage")
                nc.sync.dma_start(
                    st[:], dram[:, :].rearrange("(k p) m -> p k m", p=128)
                )
                rt = cp.tile([128, kt, shape3[2]], f32r, tag=tag)
                nc.vector.tensor_copy(rt[:], st[:])
                return rt

            w1r = load_round(d_w1, [128, 2, 1024], "w1r")
            whhr = load_round(d_whh, [128, 2, 1024], "whhr")
            wbar = load_round(d_wba, [128, 4, 512], "wbar")
            wqr = load_round(d_wq, [128, 2, 256], "wqr")
            wkr = load_round(d_wk, [128, 2, 256], "wkr")
            watr = load_round(d_wat, [128, 4, 256], "watr")
            wp1r = load_round(d_wp1, [128, 2, 128], "wp1r")
            wv1r = load_round(d_wv1, [128, 2, 64], "wv1r")

            st = sp.tile([128, 64], f32, tag="stage_s")
            nc.sync.dma_start(st[:], d_wp2[:, :])
            wp2r = cp.tile([128, 64], f32r, tag="wp2r")
            nc.vector.tensor_copy(wp2r[:], st[:])

            st = sp.tile([64, 32], f32, tag="stage_s")
            nc.sync.dma_start(st[:], d_wv2[:, :])
            wv2r = cp.tile([64, 32], f32r, tag="wv2r")
            nc.vector.tensor_copy(wv2r[:], st[:])

            st = sp.tile([96, 4], f32, tag="stage_s")
            nc.sync.dma_start(st[:], d_wpv[:, :])
            wpvr = cp.tile([96, 4], f32r, tag="wpvr")
            nc.vector.tensor_copy(wpvr[:], st[:])

            # biases (f32, per-partition scalars)
            bl = cp.tile([128, 8], f32, tag="bl")
            nc.sync.dma_start(
                bl[:], d_bl[:, :].rearrange("(m p) one -> p (m one)", p=128)
            )
            bba = cp.tile([128, 4], f32, tag="bba")
            nc.sync.dma_start(
                bba[:], d_bba[:, :].rearrange("(m p) one -> p (m one)", p=128)
            )
            bq = cp.tile([128, 2], f32, tag="bq")
            nc.sync.dma_start(
                bq[:], d_bq[:, :].rearrange("(m p) one -> p (m one)", p=128)
            )
            bk = cp.tile([128, 2], f32, tag="bk")
            nc.sync.dma_start(
                bk[:], d_bk[:, :].rearrange("(m p) one -> p (m one)", p=128)
            )
            bat = cp.tile([128, 2], f32, tag="bat")
            nc.sync.dma_start(
                bat[:], d_bat[:, :].rearrange("(m p) one -> p (m one)", p=128)
            )
            bp1 = cp.tile([128, 1], f32, tag="bp1")
            nc.sync.dma_start(bp1[:], d_bp1[:, :])
            bv1 = cp.tile([64, 1], f32, tag="bv1")
            nc.sync.dma_start(bv1[:], d_bv1[:, :])
            bp2 = cp.tile([64, 1], f32, tag="bp2")
            nc.sync.dma_start(bp2[:], d_bp2[:, :])
            bv2 = cp.tile([32, 1], f32, tag="bv2")
            nc.sync.dma_start(bv2[:], d_bv2[:, :])
            bpv = cp.tile([4, 1], f32, tag="bpv")
            nc.sync.dma_start(bpv[:], d_bpv[:, :])

            # ---------- preamble: embedding tables ----------
            ta_sb = sp.tile([26, 256], f32, tag="ta")
            nc.sync.dma_start(ta_sb[:], d_ta[:, :])
            emb_sb = sp.tile([26, 4], f32, tag="emb")
            nc.sync.dma_start(emb_sb[:], d_emb[:, :])

            # tblT_sig [4 parts, 256]: sigmoid(emb @ w_ta.T + b_ta), row-major
            tblT = cp.tile([4, 256], f32r, tag="tblT")
            ps = pp.tile([4, 256], f32, tag="ps")
            nc.tensor.matmul(ps[:], emb_sb[:], ta_sb[:], start=True, stop=True)
            nc.scalar.activation(tblT[:], ps[:], AF.Sigmoid)

            # tbl [256, 4] feature-major (2 part-tiles), f32 for the fp32
            # preamble matmul below
            tbl = []
            for ft in range(2):
                t = sp.tile([128, 4], f32, tag=f"tbl{ft}")
                ps = pp.tile([128, 4], f32, tag="ps")
                nc.tensor.matmul(
                    ps[:],
                    ta_sb[:, ft * 128 : (ft + 1) * 128],
                    emb_sb[:],
                    start=True,
                    stop=True,
                )
                nc.scalar.activation(t[:], ps[:], AF.Sigmoid)
                tbl.append(t)

            # tblg2 [4, 1024] = tbl_sig.T @ w2T  (gated_att path folded into
            # a 4-row table; fp32 matmul, preamble only)
            w2st = []
            for k in range(2):
                t = sp.tile([128, 1024], f32, tag=f"w2st{k}")
                nc.sync.dma_start(
                    t[:],
                    d_w2[:, :].rearrange("(k p) m -> p k m", p=128)[:, k, :],
                )
                w2st.append(t)
            tblg2 = cp.tile([4, 1024], f32r, tag="tblg2")
            for c in range(2):
                ps = pp.tile([4, 512], f32, tag="ps")
                nc.tensor.matmul(
                    ps[:],
                    tbl[0][:],
                    w2st[0][:, c * 512 : (c + 1) * 512],
                    start=True,
                    stop=False,
                )
                nc.tensor.matmul(
                    ps[:],
                    tbl[1][:],
                    w2st[1][:, c * 512 : (c + 1) * 512],
                    start=False,
                    stop=True,
                )
                nc.scalar.copy(tblg2[:, c * 512 : (c + 1) * 512], ps[:])

            # ---------- main loop over row blocks ----------
            for b in range(NBLK):
                r0 = b * NB
                rs = slice(r0, r0 + NB)

                x_img = ip.tile([128, 2, NB], f32, tag="x_img")
                nc.sync.dma_start(x_img[:], imgR[:, :, rs])
                x_hx = ip.tile([128, 2, NB], f32, tag="x_hx")
                nc.sync.dma_start(x_hx[:], hxR[:, :, rs])
                x_cx = ip.tile([128, 2, NB], f32, tag="x_cx")
                nc.sync.dma_start(x_cx[:], cxR[:, :, rs])
                x_q = ip.tile([128, 2, NB], f32, tag="x_q")
                nc.sync.dma_start(x_q[:], qR[:, :, rs])
                oh = ip.tile([4, NB], f32, tag="oh")
                nc.sync.dma_start(oh[:], ohR[:, rs])

                oh_r = ap2.tile([4, NB], f32r, tag="oh_r")
                nc.vector.tensor_copy(oh_r[:], oh[:])
                hx_r = ap2.tile([128, 2, NB], f32r, tag="hx_r")
                nc.vector.tensor_copy(hx_r[:], x_hx[:])
                q_r = ap2.tile([128, 2, NB], f32r, tag="q_r")
                nc.vector.tensor_copy(q_r[:], x_q[:])

                # gated_att (via table gather matmul) fused into gated_fusion
                gf_r = ap2.tile([128, 2, NB], f32r, tag="gf")
                for ft in range(2):
                    ps = pp.tile([128, NB], f32, tag="ps")
                    nc.tensor.matmul(
                        ps[:],
                        tblT[:, ft * 128 : (ft + 1) * 128],
                        oh_r[:],
                        start=True,
                        stop=True,
                    )
                    nc.vector.tensor_tensor(
                        gf_r[:, ft, :], x_img[:, ft, :], ps[:], ALU.mult
                    )

                # LSTM gates: 8 m-tiles, 5 accumulating matmuls each
                gtiles = []
                for g in range(4):
                    gt = ap_.tile([128, 2, NB], f32, tag=f"gate{g}", name=f"gate{g}")
                    gtiles.append(gt)
                for m in range(8):
                    msl = slice(m * 128, (m + 1) * 128)
                    ps = pp.tile([128, NB], f32, tag="ps")
                    nc.tensor.matmul(
                        ps[:], tblg2[:, msl], oh_r[:], start=True, stop=False
                    )
                    nc.tensor.matmul(
                        ps[:], w1r[:, 0, msl], gf_r[:, 0, :], start=False, stop=False
                    )
                    nc.tensor.matmul(
                        ps[:], w1r[:, 1, msl], gf_r[:, 1, :], start=False, stop=False
                    )
                    nc.tensor.matmul(
                        ps[:], whhr[:, 0, msl], hx_r[:, 0, :], start=False, stop=False
                    )
                    nc.tensor.matmul(
                        ps[:], whhr[:, 1, msl], hx_r[:, 1, :], start=False, stop=True
                    )
                    g = m // 2
                    func = AF.Tanh if g == 2 else AF.Sigmoid
                    nc.scalar.activation(
                        gtiles[g][:, m % 2, :], ps[:], func, bias=bl[:, m : m + 1]
                    )
                sig_i, sig_f, tanh_g, sig_o = gtiles

                # cell/hidden state update
                cx_new = ap_.tile([128, 2, NB], f32, tag="cx_new")
                hx_new = ap_.tile([128, 2, NB], f32, tag="hx_new")
                hx_new_r = ap2.tile([128, 2, NB], f32r, tag="hx_new_r")
                for ft in range(2):
                    t_ig = tp.tile([128, NB], f32, tag="tmp")
                    nc.vector.tensor_tensor(
                        t_ig[:], sig_i[:, ft, :], tanh_g[:, ft, :], ALU.mult
                    )
                    t_fc = tp.tile([128, NB], f32, tag="tmp")
                    nc.vector.tensor_tensor(
                        t_fc[:], sig_f[:, ft, :], x_cx[:, ft, :], ALU.mult
                    )
                    nc.vector.tensor_tensor(
                        cx_new[:, ft, :], t_ig[:], t_fc[:], ALU.add
                    )
                    tcx = tp.tile([128, NB], f32, tag="tmp")
                    nc.scalar.activation(tcx[:], cx_new[:, ft, :], AF.Tanh)
                    nc.vector.tensor_tensor(
                        hx_new[:, ft, :], sig_o[:, ft, :], tcx[:], ALU.mult
                    )
                nc.vector.tensor_copy(hx_new_r[:], hx_new[:])
                nc.sync.dma_start(hxoR[:, :, rs], hx_new[:])
                nc.sync.dma_start(cxoR[:, :, rs], cx_new[:])

                # mlp_attn = relu([gf; hx] @ w_ba.T + b_ba) -> [key; val]
                ma = ap2.tile([128, 4, NB], f32r, tag="ma", bufs=1)
                for m in range(4):
                    msl = slice(m * 128, (m + 1) * 128)
                    ps = pp.tile([128, NB], f32, tag="ps")
                    nc.tensor.matmul(
                        ps[:], wbar[:, 0, msl], gf_r[:, 0, :], start=True, stop=False
                    )
                    nc.tensor.matmul(
                        ps[:], wbar[:, 1, msl], gf_r[:, 1, :], start=False, stop=False
                    )
                    nc.tensor.matmul(
                        ps[:], wbar[:, 2, msl], hx_new_r[:, 0, :],
                        start=False, stop=False,
                    )
                    nc.tensor.matmul(
                        ps[:], wbar[:, 3, msl], hx_new_r[:, 1, :],
                        start=False, stop=True,
                    )
                    nc.scalar.activation(
                        ma[:, m, :], ps[:], AF.Relu, bias=bba[:, m : m + 1]
                    )

                # query / key projections (query path independent of LSTM)
                wqo = ap_.tile([128, 2, NB], f32, tag="wqo")
                wko = ap_.tile([128, 2, NB], f32, tag="wko")
                for m in range(2):
                    msl = slice(m * 128, (m + 1) * 128)
                    ps = pp.tile([128, NB], f32, tag="ps")
                    nc.tensor.matmul(
                        ps[:], wqr[:, 0, msl], q_r[:, 0, :], start=True, stop=False
                    )
                    nc.tensor.matmul(
                        ps[:], wqr[:, 1, msl], q_r[:, 1, :], start=False, stop=True
                    )
                    nc.scalar.activation(
                        wqo[:, m, :], ps[:], AF.Relu, bias=bq[:, m : m + 1]
                    )
                for m in range(2):
                    msl = slice(m * 128, (m + 1) * 128)
                    ps = pp.tile([128, NB], f32, tag="ps")
                    nc.tensor.matmul(
                        ps[:], wkr[:, 0, msl], ma[:, 0, :], start=True, stop=False
                    )
                    nc.tensor.matmul(
                        ps[:], wkr[:, 1, msl], ma[:, 1, :], start=False, stop=True
                    )
                    nc.scalar.activation(
                        wko[:, m, :], ps[:], AF.Relu, bias=bk[:, m : m + 1]
                    )

                # u_t = tanh(wq+wk); attention_vector = u_t * val
                attv_r = ap2.tile([128, 2, NB], f32r, tag="attv", bufs=1)
                for ft in range(2):
                    us = tp.tile([128, NB], f32, tag="tmp")
                    nc.vector.tensor_tensor(
                        us[:], wqo[:, ft, :], wko[:, ft, :], ALU.add
                    )
                    ut = tp.tile([128, NB], f32, tag="tmp")
                    nc.scalar.activation(ut[:], us[:], AF.Tanh)
                    nc.vector.tensor_tensor(
                        attv_r[:, ft, :],
                        ut[:],
                        ma[:, 2 + ft, :].bitcast(f32),
                        ALU.mult,
                    )

                # attn_weight = relu([attv; hx] @ w_at.T + b_at)
                ao = ap2.tile([128, 2, NB], f32r, tag="ao", bufs=1)
                for m in range(2):
                    msl = slice(m * 128, (m + 1) * 128)
                    ps = pp.tile([128, NB], f32, tag="ps")
                    nc.tensor.matmul(
                        ps[:], watr[:, 0, msl], attv_r[:, 0, :],
                        start=True, stop=False,
                    )
                    nc.tensor.matmul(
                        ps[:], watr[:, 1, msl], attv_r[:, 1, :],
                        start=False, stop=False,
                    )
                    nc.tensor.matmul(
                        ps[:], watr[:, 2, msl], hx_new_r[:, 0, :],
                        start=False, stop=False,
                    )
                    nc.tensor.matmul(
                        ps[:], watr[:, 3, msl], hx_new_r[:, 1, :],
                        start=False, stop=True,
                    )
                    nc.scalar.activation(
                        ao[:, m, :], ps[:], AF.Relu, bias=bat[:, m : m + 1]
                    )

                # policy / value heads
                p1 = ap_.tile([128, NB], f32r, tag="p1")
                ps = pp.tile([128, NB], f32, tag="ps")
                nc.tensor.matmul(
                    ps[:], wp1r[:, 0, :], ao[:, 0, :], start=True, stop=False
                )
                nc.tensor.matmul(
                    ps[:], wp1r[:, 1, :], ao[:, 1, :], start=False, stop=True
                )
                nc.scalar.activation(p1[:], ps[:], AF.Relu, bias=bp1[:, 0:1])

                v1 = ap_.tile([64, NB], f32r, tag="v1")
                ps = pp.tile([64, NB], f32, tag="ps")
                nc.tensor.matmul(
                    ps[:], wv1r[:, 0, :], ao[:, 0, :], start=True, stop=False
                )
                nc.tensor.matmul(
                    ps[:], wv1r[:, 1, :], ao[:, 1, :], start=False, stop=True
                )
                nc.scalar.activation(v1[:], ps[:], AF.Relu, bias=bv1[:, 0:1])

                pv_in = ap_.tile([96, NB], f32r, tag="pv_in")
                ps = pp.tile([64, NB], f32, tag="ps")
                nc.tensor.matmul(ps[:], wp2r[:], p1[:], start=True, stop=True)
                nc.scalar.activation(
                    pv_in[0:64, :], ps[:], AF.Relu, bias=bp2[:, 0:1]
                )
                ps = pp.tile([32, NB], f32, tag="ps")
                nc.tensor.matmul(ps[:], wv2r[:], v1[:], start=True, stop=True)
                nc.scalar.activation(
                    pv_in[64:96, :], ps[:], AF.Relu, bias=bv2[:, 0:1]
                )

                outpv = ap_.tile([4, NB], f32, tag="outpv")
                ps = pp.tile([4, NB], f32, tag="ps")
                nc.tensor.matmul(ps[:], wpvr[:], pv_in[:], start=True, stop=True)
                nc.scalar.activation(
                    outpv[:], ps[:], AF.Identity, bias=bpv[:, 0:1]
                )
                nc.sync.dma_start(pvoR[:, rs], outpv[:])

    _fix_excess_waits(nc, mybir)
    return nc


def _prep_params(inputs):
    f = np.float32

    def c(x):
        return np.ascontiguousarray(x, dtype=f)

    w_ih = np.asarray(inputs["w_ih"], f)
    p = {
        "ta_ext": c(
            np.concatenate(
                [np.asarray(inputs["w_ta"], f).T, np.asarray(inputs["b_ta"], f)[None, :]], 0
            )
        ),
        "emb_ext": c(
            np.concatenate(
                [np.asarray(inputs["emb"], f).T, np.ones((1, 4), f)], 0
            )
        ),
        "w1T": c(w_ih[:, :256].T),
        "w2T": c(w_ih[:, 256:].T),
        "whhT": c(np.asarray(inputs["w_hh"], f).T),
        "b_lstm": c(
            (np.asarray(inputs["b_ih"], f) + np.asarray(inputs["b_hh"], f)).reshape(
                1024, 1
            )
        ),
        "wbaT": c(np.asarray(inputs["w_ba"], f).T),
        "b_ba": c(np.asarray(inputs["b_ba"], f).reshape(512, 1)),
        "wqT": c(np.asarray(inputs["w_q"], f).T),
        "b_q": c(np.asarray(inputs["b_q"], f).reshape(256, 1)),
        "wkT": c(np.asarray(inputs["w_k"], f).T),
        "b_k": c(np.asarray(inputs["b_k"], f).reshape(256, 1)),
        "watT": c(np.asarray(inputs["w_at"], f).T),
        "b_at": c(np.asarray(inputs["b_at"], f).reshape(256, 1)),
        "wp1T": c(np.asarray(inputs["w_p1"], f).T),
        "b_p1": c(np.asarray(inputs["b_p1"], f).reshape(128, 1)),
        "wv1T": c(np.asarray(inputs["w_v1"], f).T),
        "b_v1": c(np.asarray(inputs["b_v1"], f).reshape(64, 1)),
        "wp2T": c(np.asarray(inputs["w_p2"], f).T),
        "b_p2": c(np.asarray(inputs["b_p2"], f).reshape(64, 1)),
        "wv2T": c(np.asarray(inputs["w_v2"], f).T),
        "b_v2": c(np.asarray(inputs["b_v2"], f).reshape(32, 1)),
    }
    wpv = np.zeros((96, 4), f)
    wpv[0:64, 0:3] = np.asarray(inputs["w_p"], f).T
    wpv[64:96, 3:4] = np.asarray(inputs["w_v"], f).T
    p["wpvT"] = wpv
    p["b_pv"] = c(
        np.concatenate(
            [np.asarray(inputs["b_p"], f).ravel(), np.asarray(inputs["b_v"], f).ravel()]
        ).reshape(4, 1)
    )
    return p


LAST_RESULT = None


def kernel(**inputs):
    global LAST_RESULT
    from concourse.bass_utils import run_bass_kernel_spmd

    if "nc" not in _CACHED:
        _CACHED["nc"] = _build()
    nc = _CACHED["nc"]

    f = np.float32
    img = np.asarray(inputs["img_feat"], f)
    hx = np.asarray(inputs["_hx"], f)
    cx = np.asarray(inputs["_cx"], f)
    q = np.asarray(inputs["query"], f)
    idx = np.asarray(inputs["instruction_idx"]).reshape(-1).astype(np.int64)

    params = _prep_params(inputs)

    in_maps = []
    for core in range(NCORES):
        rows = slice(core * R, (core + 1) * R)
        oh = (idx[rows][None, :] == np.arange(4)[:, None]).astype(f)
        m = {
            "imgT": np.ascontiguousarray(img[rows].T),
            "hxT": np.ascontiguousarray(hx[rows].T),
            "cxT": np.ascontiguousarray(cx[rows].T),
            "queryT": np.ascontiguousarray(q[rows].T),
            "ohT": np.ascontiguousarray(oh),
        }
        m.update(params)
        in_maps.append(m)

    res = run_bass_kernel_spmd(nc, in_maps, core_ids=list(range(NCORES)))
    LAST_RESULT = res

    hx_out = np.empty((B, 256), f)
    cx_out = np.empty((B, 256), f)
    val_out = np.empty((B, 1), f)
    pol_out = np.empty((B, 3), f)
    for core in range(NCORES):
        rows = slice(core * R, (core + 1) * R)
        r = res.results[core]
        hx_out[rows] = r["hxT_out"].T
        cx_out[rows] = r["cxT_out"].T
        pol_out[rows] = r["pvT_out"][0:3].T
        val_out[rows] = r["pvT_out"][3:4].T
    return (val_out, pol_out, hx_out, cx_out)


# revision 7
# speedup vs baseline: 1.3246x; 1.3246x over previous
"""Trainium2 Bass kernel for nn_Actor_Critic (dense MLP + LSTM cell + attention).

Strategy: pure data parallel over 8 NeuronCores. Each core processes
B/8 = 4096 rows with replicated (tiny) weights.

Layout: activations are feature-major on chip ([feature_part, row_free]),
so every layer's matmul is out = W_T.T @ x_T with the contraction on the
partition dim, biases become per-partition scalars fused into the ACT
eviction, and the output stays feature-major for the next layer.
Host transposes the [B,256] inputs once and transposes outputs back.

Compute dtype: bf16 for matmul operands (full PE rate, fast weight load,
background LDWEIGHTS overlap - fp32/f32r matmuls must self-load weights
serially, which measured ~262ns/matmul of pure PE stall). State math and
outputs stay f32; producers convert on their output write for free.

Embedding folding: instruction_idx has only 4 values, so
  gated_att = sigmoid(emb[idx] @ w_ta.T + b_ta) = table4[:, idx]
and the gated_att @ w_ih[:,256:512].T term of the LSTM gates is
  onehot.T @ (table4_sig @ w2.T)  -- a [4,1024] table; both tables are
built on-device in a tiny preamble. This removes a K=256 matmul per layer
chain (~20% of total PE work).
"""

import numpy as np

B = 32768
NCORES = 8
R = B // NCORES          # rows per core = 4096
NB = 512                 # rows per moving block (fp32 moving max / PSUM bank)
NBLK = R // NB           # 8 blocks per core

_CACHED = {}


def _fix_excess_waits(nc, mybir, keep=1, per_nop=1):
    """This container's walrus accepts only a limited number of sync-wait
    commands per instruction, but Tile can emit more (observed on the tail
    drain and on matmuls). Hoist excess waits onto carrier NoOps inserted
    just before the over-limit instruction on the same engine, preserving
    per-engine program order."""
    n_fixed = 0
    for f in nc.m.functions:
        for bb in f.blocks:
            insts = list(bb.instructions)
            out = []
            changed = False
            for inst in insts:
                si = inst.sync_info
                waits = list(si.on_wait) if si is not None and si.on_wait else []
                if len(waits) > keep:
                    excess, rest = waits[:-keep], waits[-keep:]
                    for i in range(0, len(excess), per_nop):
                        nop = mybir.InstNoOp(
                            name=nc.get_next_instruction_name(), ins=[], outs=[]
                        )
                        nop.engine = inst.engine
                        nop.sync_info = mybir.SyncInfo(
                            on_wait=excess[i : i + per_nop], on_update=[]
                        )
                        out.append(nop)
                    si.on_wait = rest
                    changed = True
                    n_fixed += 1
                out.append(inst)
            if changed:
                bb.instructions = out
    return n_fixed


def _build():
    import concourse.bass as bass
    import concourse.mybir as mybir
    from concourse.tile import TileContext

    f32 = mybir.dt.float32
    bf16 = mybir.dt.bfloat16
    AF = mybir.ActivationFunctionType
    ALU = mybir.AluOpType

    nc = bass.Bass("TRN2")

    # ---- DRAM parameters (per-core shard views; host pre-transposed) ----
    d_img = nc.declare_dram_parameter("imgT", [256, R], f32, isOutput=False)
    d_hx = nc.declare_dram_parameter("hxT", [256, R], f32, isOutput=False)
    d_cx = nc.declare_dram_parameter("cxT", [256, R], f32, isOutput=False)
    d_q = nc.declare_dram_parameter("queryT", [256, R], f32, isOutput=False)
    d_oh = nc.declare_dram_parameter("ohT", [4, R], f32, isOutput=False)

    d_ta = nc.declare_dram_parameter("ta_ext", [26, 256], f32, isOutput=False)
    d_emb = nc.declare_dram_parameter("emb_ext", [26, 4], f32, isOutput=False)
    d_w1 = nc.declare_dram_parameter("w1T", [256, 1024], f32, isOutput=False)
    d_w2 = nc.declare_dram_parameter("w2T", [256, 1024], f32, isOutput=False)
    d_whh = nc.declare_dram_parameter("whhT", [256, 1024], f32, isOutput=False)
    d_bl = nc.declare_dram_parameter("b_lstm", [1024, 1], f32, isOutput=False)
    d_wba = nc.declare_dram_parameter("wbaT", [512, 512], f32, isOutput=False)
    d_bba = nc.declare_dram_parameter("b_ba", [512, 1], f32, isOutput=False)
    d_wq = nc.declare_dram_parameter("wqT", [256, 256], f32, isOutput=False)
    d_bq = nc.declare_dram_parameter("b_q", [256, 1], f32, isOutput=False)
    d_wk = nc.declare_dram_parameter("wkT", [256, 256], f32, isOutput=False)
    d_bk = nc.declare_dram_parameter("b_k", [256, 1], f32, isOutput=False)
    d_wat = nc.declare_dram_parameter("watT", [512, 256], f32, isOutput=False)
    d_bat = nc.declare_dram_parameter("b_at", [256, 1], f32, isOutput=False)
    d_wp1 = nc.declare_dram_parameter("wp1T", [256, 128], f32, isOutput=False)
    d_bp1 = nc.declare_dram_parameter("b_p1", [128, 1], f32, isOutput=False)
    d_wv1 = nc.declare_dram_parameter("wv1T", [256, 64], f32, isOutput=False)
    d_bv1 = nc.declare_dram_parameter("b_v1", [64, 1], f32, isOutput=False)
    d_wp2 = nc.declare_dram_parameter("wp2T", [128, 64], f32, isOutput=False)
    d_bp2 = nc.declare_dram_parameter("b_p2", [64, 1], f32, isOutput=False)
    d_wv2 = nc.declare_dram_parameter("wv2T", [64, 32], f32, isOutput=False)
    d_bv2 = nc.declare_dram_parameter("b_v2", [32, 1], f32, isOutput=False)
    d_wpv = nc.declare_dram_parameter("wpvT", [96, 4], f32, isOutput=False)
    d_bpv = nc.declare_dram_parameter("b_pv", [4, 1], f32, isOutput=False)

    d_hxo = nc.declare_dram_parameter("hxT_out", [256, R], f32, isOutput=True)
    d_cxo = nc.declare_dram_parameter("cxT_out", [256, R], f32, isOutput=True)
    d_pvo = nc.declare_dram_parameter("pvT_out", [4, R], f32, isOutput=True)

    # feature-major DRAM views split into 128-partition k-tiles
    imgR = d_img[:, :].rearrange("(k p) r -> p k r", p=128)
    hxR = d_hx[:, :].rearrange("(k p) r -> p k r", p=128)
    cxR = d_cx[:, :].rearrange("(k p) r -> p k r", p=128)
    qR = d_q[:, :].rearrange("(k p) r -> p k r", p=128)
    ohR = d_oh[:, :]
    hxoR = d_hxo[:, :].rearrange("(k p) r -> p k r", p=128)
    cxoR = d_cxo[:, :].rearrange("(k p) r -> p k r", p=128)
    pvoR = d_pvo[:, :]

    with TileContext(nc) as tc:
        with (
            tc.tile_pool(name="const", bufs=1) as cp,
            tc.tile_pool(name="stage", bufs=1) as sp,
            tc.tile_pool(name="inp", bufs=2) as ip,
            tc.tile_pool(name="act", bufs=1) as ap_,
            tc.tile_pool(name="act2", bufs=2) as ap2,
            tc.tile_pool(name="tmp", bufs=6) as tp,
            tc.tile_pool(name="ps", bufs=8, space="PSUM") as pp,
        ):
            # ---------- weights: DMA + round to f32r ----------
            def load_round(dram, shape3, tag):
                """DMA [K, M] weight as [128, kt, M] tiles and round to f32r."""
                kt = shape3[1]
                st = sp.tile([128, kt, shape3[2]], f32, tag="stage")
                nc.sync.dma_start(
                    st[:], dram[:, :].rearrange("(k p) m -> p k m", p=128)
                )
                rt = cp.tile([128, kt, shape3[2]], bf16, tag=tag)
                nc.vector.tensor_copy(rt[:], st[:])
                return rt

            w1r = load_round(d_w1, [128, 2, 1024], "w1r")
            whhr = load_round(d_whh, [128, 2, 1024], "whhr")
            wbar = load_round(d_wba, [128, 4, 512], "wbar")
            wqr = load_round(d_wq, [128, 2, 256], "wqr")
            wkr = load_round(d_wk, [128, 2, 256], "wkr")
            watr = load_round(d_wat, [128, 4, 256], "watr")
            wp1r = load_round(d_wp1, [128, 2, 128], "wp1r")
            wv1r = load_round(d_wv1, [128, 2, 64], "wv1r")

            st = sp.tile([128, 64], f32, tag="stage_s")
            nc.sync.dma_start(st[:], d_wp2[:, :])
            wp2r = cp.tile([128, 64], bf16, tag="wp2r")
            nc.vector.tensor_copy(wp2r[:], st[:])

            st = sp.tile([64, 32], f32, tag="stage_s")
            nc.sync.dma_start(st[:], d_wv2[:, :])
            wv2r = cp.tile([64, 32], bf16, tag="wv2r")
            nc.vector.tensor_copy(wv2r[:], st[:])

            st = sp.tile([96, 4], f32, tag="stage_s")
            nc.sync.dma_start(st[:], d_wpv[:, :])
            wpvr = cp.tile([96, 4], bf16, tag="wpvr")
            nc.vector.tensor_copy(wpvr[:], st[:])

            # biases (f32, per-partition scalars)
            bl = cp.tile([128, 8], f32, tag="bl")
            nc.sync.dma_start(
                bl[:], d_bl[:, :].rearrange("(m p) one -> p (m one)", p=128)
            )
            bba = cp.tile([128, 4], f32, tag="bba")
            nc.sync.dma_start(
                bba[:], d_bba[:, :].rearrange("(m p) one -> p (m one)", p=128)
            )
            bq = cp.tile([128, 2], f32, tag="bq")
            nc.sync.dma_start(
                bq[:], d_bq[:, :].rearrange("(m p) one -> p (m one)", p=128)
            )
            bk = cp.tile([128, 2], f32, tag="bk")
            nc.sync.dma_start(
                bk[:], d_bk[:, :].rearrange("(m p) one -> p (m one)", p=128)
            )
            bat = cp.tile([128, 2], f32, tag="bat")
            nc.sync.dma_start(
                bat[:], d_bat[:, :].rearrange("(m p) one -> p (m one)", p=128)
            )
            bp1 = cp.tile([128, 1], f32, tag="bp1")
            nc.sync.dma_start(bp1[:], d_bp1[:, :])
            bv1 = cp.tile([64, 1], f32, tag="bv1")
            nc.sync.dma_start(bv1[:], d_bv1[:, :])
            bp2 = cp.tile([64, 1], f32, tag="bp2")
            nc.sync.dma_start(bp2[:], d_bp2[:, :])
            bv2 = cp.tile([32, 1], f32, tag="bv2")
            nc.sync.dma_start(bv2[:], d_bv2[:, :])
            bpv = cp.tile([4, 1], f32, tag="bpv")
            nc.sync.dma_start(bpv[:], d_bpv[:, :])

            # ---------- preamble: embedding tables ----------
            ta_sb = sp.tile([26, 256], f32, tag="ta")
            nc.sync.dma_start(ta_sb[:], d_ta[:, :])
            emb_sb = sp.tile([26, 4], f32, tag="emb")
            nc.sync.dma_start(emb_sb[:], d_emb[:, :])

            # tblT_sig [4 parts, 256]: sigmoid(emb @ w_ta.T + b_ta), row-major
            tblT = cp.tile([4, 256], bf16, tag="tblT")
            ps = pp.tile([4, 256], f32, tag="ps")
            nc.tensor.matmul(ps[:], emb_sb[:], ta_sb[:], start=True, stop=True)
            nc.scalar.activation(tblT[:], ps[:], AF.Sigmoid)

            # tbl [256, 4] feature-major (2 part-tiles), f32 for the fp32
            # preamble matmul below
            tbl = []
            for ft in range(2):
                t = sp.tile([128, 4], f32, tag=f"tbl{ft}")
                ps = pp.tile([128, 4], f32, tag="ps")
                nc.tensor.matmul(
                    ps[:],
                    ta_sb[:, ft * 128 : (ft + 1) * 128],
                    emb_sb[:],
                    start=True,
                    stop=True,
                )
                nc.scalar.activation(t[:], ps[:], AF.Sigmoid)
                tbl.append(t)

            # tblg2 [4, 1024] = tbl_sig.T @ w2T  (gated_att path folded into
            # a 4-row table; fp32 matmul, preamble only)
            w2st = []
            for k in range(2):
                t = sp.tile([128, 1024], f32, tag=f"w2st{k}")
                nc.sync.dma_start(
                    t[:],
                    d_w2[:, :].rearrange("(k p) m -> p k m", p=128)[:, k, :],
                )
                w2st.append(t)
            tblg2 = cp.tile([4, 1024], bf16, tag="tblg2")
            for c in range(2):
                ps = pp.tile([4, 512], f32, tag="ps")
                nc.tensor.matmul(
                    ps[:],
                    tbl[0][:],
                    w2st[0][:, c * 512 : (c + 1) * 512],
                    start=True,
                    stop=False,
                )
                nc.tensor.matmul(
                    ps[:],
                    tbl[1][:],
                    w2st[1][:, c * 512 : (c + 1) * 512],
                    start=False,
                    stop=True,
                )
                nc.scalar.copy(tblg2[:, c * 512 : (c + 1) * 512], ps[:])

            # ---------- main loop over row blocks ----------
            for b in range(NBLK):
                r0 = b * NB
                rs = slice(r0, r0 + NB)

                x_img = ip.tile([128, 2, NB], f32, tag="x_img")
                nc.sync.dma_start(x_img[:], imgR[:, :, rs])
                x_hx = ip.tile([128, 2, NB], f32, tag="x_hx")
                nc.sync.dma_start(x_hx[:], hxR[:, :, rs])
                x_cx = ip.tile([128, 2, NB], f32, tag="x_cx")
                nc.sync.dma_start(x_cx[:], cxR[:, :, rs])
                x_q = ip.tile([128, 2, NB], f32, tag="x_q")
                nc.sync.dma_start(x_q[:], qR[:, :, rs])
                oh = ip.tile([4, NB], f32, tag="oh")
                nc.sync.dma_start(oh[:], ohR[:, rs])

                oh_r = ap2.tile([4, NB], bf16, tag="oh_r")
                nc.vector.tensor_copy(oh_r[:], oh[:])
                hx_r = ap2.tile([128, 2, NB], bf16, tag="hx_r")
                nc.vector.tensor_copy(hx_r[:], x_hx[:])
                q_r = ap2.tile([128, 2, NB], bf16, tag="q_r")
                nc.vector.tensor_copy(q_r[:], x_q[:])

                # gated_att (via table gather matmul) fused into gated_fusion
                gf_r = ap2.tile([128, 2, NB], bf16, tag="gf")
                for ft in range(2):
                    ps = pp.tile([128, NB], f32, tag="ps")
                    nc.tensor.matmul(
                        ps[:],
                        tblT[:, ft * 128 : (ft + 1) * 128],
                        oh_r[:],
                        start=True,
                        stop=True,
                    )
                    nc.vector.tensor_tensor(
                        gf_r[:, ft, :], x_img[:, ft, :], ps[:], ALU.mult
                    )

                # LSTM gates: 8 m-tiles, 5 accumulating matmuls each
                gtiles = []
                for g in range(4):
                    gt = ap_.tile([128, 2, NB], f32, tag=f"gate{g}", name=f"gate{g}")
                    gtiles.append(gt)
                for m in range(8):
                    msl = slice(m * 128, (m + 1) * 128)
                    ps = pp.tile([128, NB], f32, tag="ps")
                    nc.tensor.matmul(
                        ps[:], tblg2[:, msl], oh_r[:], start=True, stop=False
                    )
                    nc.tensor.matmul(
                        ps[:], w1r[:, 0, msl], gf_r[:, 0, :], start=False, stop=False
                    )
                    nc.tensor.matmul(
                        ps[:], w1r[:, 1, msl], gf_r[:, 1, :], start=False, stop=False
                    )
                    nc.tensor.matmul(
                        ps[:], whhr[:, 0, msl], hx_r[:, 0, :], start=False, stop=False
                    )
                    nc.tensor.matmul(
                        ps[:], whhr[:, 1, msl], hx_r[:, 1, :], start=False, stop=True
                    )
                    g = m // 2
                    func = AF.Tanh if g == 2 else AF.Sigmoid
                    nc.scalar.activation(
                        gtiles[g][:, m % 2, :], ps[:], func, bias=bl[:, m : m + 1]
                    )
                sig_i, sig_f, tanh_g, sig_o = gtiles

                # cell/hidden state update
                cx_new = ap_.tile([128, 2, NB], f32, tag="cx_new")
                hx_new = ap_.tile([128, 2, NB], f32, tag="hx_new")
                hx_new_r = ap2.tile([128, 2, NB], bf16, tag="hx_new_r")
                for ft in range(2):
                    t_ig = tp.tile([128, NB], f32, tag="tmp")
                    nc.vector.tensor_tensor(
                        t_ig[:], sig_i[:, ft, :], tanh_g[:, ft, :], ALU.mult
                    )
                    t_fc = tp.tile([128, NB], f32, tag="tmp")
                    nc.vector.tensor_tensor(
                        t_fc[:], sig_f[:, ft, :], x_cx[:, ft, :], ALU.mult
                    )
                    nc.vector.tensor_tensor(
                        cx_new[:, ft, :], t_ig[:], t_fc[:], ALU.add
                    )
                    tcx = tp.tile([128, NB], f32, tag="tmp")
                    nc.scalar.activation(tcx[:], cx_new[:, ft, :], AF.Tanh)
                    nc.vector.tensor_tensor(
                        hx_new[:, ft, :], sig_o[:, ft, :], tcx[:], ALU.mult
                    )
                nc.vector.tensor_copy(hx_new_r[:], hx_new[:])
                nc.sync.dma_start(hxoR[:, :, rs], hx_new[:])
                nc.sync.dma_start(cxoR[:, :, rs], cx_new[:])

                # mlp_attn = relu([gf; hx] @ w_ba.T + b_ba) -> [key; val]
                ma = ap2.tile([128, 4, NB], bf16, tag="ma", bufs=1)
                for m in range(4):
                    msl = slice(m * 128, (m + 1) * 128)
                    ps = pp.tile([128, NB], f32, tag="ps")
                    nc.tensor.matmul(
                        ps[:], wbar[:, 0, msl], gf_r[:, 0, :], start=True, stop=False
                    )
                    nc.tensor.matmul(
                        ps[:], wbar[:, 1, msl], gf_r[:, 1, :], start=False, stop=False
                    )
                    nc.tensor.matmul(
                        ps[:], wbar[:, 2, msl], hx_new_r[:, 0, :],
                        start=False, stop=False,
                    )
                    nc.tensor.matmul(
                        ps[:], wbar[:, 3, msl], hx_new_r[:, 1, :],
                        start=False, stop=True,
                    )
                    nc.scalar.activation(
                        ma[:, m, :], ps[:], AF.Relu, bias=bba[:, m : m + 1]
                    )

                # query / key projections (query path independent of LSTM)
                wqo = ap_.tile([128, 2, NB], f32, tag="wqo")
                wko = ap_.tile([128, 2, NB], f32, tag="wko")
                for m in range(2):
                    msl = slice(m * 128, (m + 1) * 128)
                    ps = pp.tile([128, NB], f32, tag="ps")
                    nc.tensor.matmul(
                        ps[:], wqr[:, 0, msl], q_r[:, 0, :], start=True, stop=False
                    )
                    nc.tensor.matmul(
                        ps[:], wqr[:, 1, msl], q_r[:, 1, :], start=False, stop=True
                    )
                    nc.scalar.activation(
                        wqo[:, m, :], ps[:], AF.Relu, bias=bq[:, m : m + 1]
                    )
                for m in range(2):
                    msl = slice(m * 128, (m + 1) * 128)
                    ps = pp.tile([128, NB], f32, tag="ps")
                    nc.tensor.matmul(
                        ps[:], wkr[:, 0, msl], ma[:, 0, :], start=True, stop=False
                    )
                    nc.tensor.matmul(
                        ps[:], wkr[:, 1, msl], ma[:, 1, :], start=False, stop=True
                    )
                    nc.scalar.activation(
                        wko[:, m, :], ps[:], AF.Relu, bias=bk[:, m : m + 1]
                    )

                # u_t = tanh(wq+wk); attention_vector = u_t * val
                attv_r = ap2.tile([128, 2, NB], bf16, tag="attv", bufs=1)
                for ft in range(2):
                    us = tp.tile([128, NB], f32, tag="tmp")
                    nc.vector.tensor_tensor(
                        us[:], wqo[:, ft, :], wko[:, ft, :], ALU.add
                    )
                    ut = tp.tile([128, NB], f32, tag="tmp")
                    nc.scalar.activation(ut[:], us[:], AF.Tanh)
                    nc.vector.tensor_tensor(
                        attv_r[:, ft, :],
                        ut[:],
                        ma[:, 2 + ft, :],
                        ALU.mult,
                    )

                # attn_weight = relu([attv; hx] @ w_at.T + b_at)
                ao = ap2.tile([128, 2, NB], bf16, tag="ao", bufs=1)
                for m in range(2):
                    msl = slice(m * 128, (m + 1) * 128)
                    ps = pp.tile([128, NB], f32, tag="ps")
                    nc.tensor.matmul(
                        ps[:], watr[:, 0, msl], attv_r[:, 0, :],
                        start=True, stop=False,
                    )
                    nc.tensor.matmul(
                        ps[:], watr[:, 1, msl], attv_r[:, 1, :],
                        start=False, stop=False,
                    )
                    nc.tensor.matmul(
                        ps[:], watr[:, 2, msl], hx_new_r[:, 0, :],
                        start=False, stop=False,
                    )
                    nc.tensor.matmul(
                        ps[:], watr[:, 3, msl], hx_new_r[:, 1, :],
                        start=False, stop=True,
                    )
                    nc.scalar.activation(
                        ao[:, m, :], ps[:], AF.Relu, bias=bat[:, m : m + 1]
                    )

                # policy / value heads
                p1 = ap_.tile([128, NB], bf16, tag="p1")
                ps = pp.tile([128, NB], f32, tag="ps")
                nc.tensor.matmul(
                    ps[:], wp1r[:, 0, :], ao[:, 0, :], start=True, stop=False
                )
                nc.tensor.matmul(
                    ps[:], wp1r[:, 1, :], ao[:, 1, :], start=False, stop=True
                )
                nc.scalar.activation(p1[:], ps[:], AF.Relu, bias=bp1[:, 0:1])

                v1 = ap_.tile([64, NB], bf16, tag="v1")
                ps = pp.tile([64, NB], f32, tag="ps")
                nc.tensor.matmul(
                    ps[:], wv1r[:, 0, :], ao[:, 0, :], start=True, stop=False
                )
                nc.tensor.matmul(
                    ps[:], wv1r[:, 1, :], ao[:, 1, :], start=False, stop=True
                )
                nc.scalar.activation(v1[:], ps[:], AF.Relu, bias=bv1[:, 0:1])

                pv_in = ap_.tile([96, NB], bf16, tag="pv_in")
                ps = pp.tile([64, NB], f32, tag="ps")
                nc.tensor.matmul(ps[:], wp2r[:], p1[:], start=True, stop=True)
                nc.scalar.activation(
                    pv_in[0:64, :], ps[:], AF.Relu, bias=bp2[:, 0:1]
                )
                ps = pp.tile([32, NB], f32, tag="ps")
                nc.tensor.matmul(ps[:], wv2r[:], v1[:], start=True, stop=True)
                nc.scalar.activation(
                    pv_in[64:96, :], ps[:], AF.Relu, bias=bv2[:, 0:1]
                )

                outpv = ap_.tile([4, NB], f32, tag="outpv")
                ps = pp.tile([4, NB], f32, tag="ps")
                nc.tensor.matmul(ps[:], wpvr[:], pv_in[:], start=True, stop=True)
                nc.scalar.activation(
                    outpv[:], ps[:], AF.Identity, bias=bpv[:, 0:1]
                )
                nc.sync.dma_start(pvoR[:, rs], outpv[:])

    _fix_excess_waits(nc, mybir)
    return nc


def _prep_params(inputs):
    f = np.float32

    def c(x):
        return np.ascontiguousarray(x, dtype=f)

    w_ih = np.asarray(inputs["w_ih"], f)
    p = {
        "ta_ext": c(
            np.concatenate(
                [np.asarray(inputs["w_ta"], f).T, np.asarray(inputs["b_ta"], f)[None, :]], 0
            )
        ),
        "emb_ext": c(
            np.concatenate(
                [np.asarray(inputs["emb"], f).T, np.ones((1, 4), f)], 0
            )
        ),
        "w1T": c(w_ih[:, :256].T),
        "w2T": c(w_ih[:, 256:].T),
        "whhT": c(np.asarray(inputs["w_hh"], f).T),
        "b_lstm": c(
            (np.asarray(inputs["b_ih"], f) + np.asarray(inputs["b_hh"], f)).reshape(
                1024, 1
            )
        ),
        "wbaT": c(np.asarray(inputs["w_ba"], f).T),
        "b_ba": c(np.asarray(inputs["b_ba"], f).reshape(512, 1)),
        "wqT": c(np.asarray(inputs["w_q"], f).T),
        "b_q": c(np.asarray(inputs["b_q"], f).reshape(256, 1)),
        "wkT": c(np.asarray(inputs["w_k"], f).T),
        "b_k": c(np.asarray(inputs["b_k"], f).reshape(256, 1)),
        "watT": c(np.asarray(inputs["w_at"], f).T),
        "b_at": c(np.asarray(inputs["b_at"], f).reshape(256, 1)),
        "wp1T": c(np.asarray(inputs["w_p1"], f).T),
        "b_p1": c(np.asarray(inputs["b_p1"], f).reshape(128, 1)),
        "wv1T": c(np.asarray(inputs["w_v1"], f).T),
        "b_v1": c(np.asarray(inputs["b_v1"], f).reshape(64, 1)),
        "wp2T": c(np.asarray(inputs["w_p2"], f).T),
        "b_p2": c(np.asarray(inputs["b_p2"], f).reshape(64, 1)),
        "wv2T": c(np.asarray(inputs["w_v2"], f).T),
        "b_v2": c(np.asarray(inputs["b_v2"], f).reshape(32, 1)),
    }
    wpv = np.zeros((96, 4), f)
    wpv[0:64, 0:3] = np.asarray(inputs["w_p"], f).T
    wpv[64:96, 3:4] = np.asarray(inputs["w_v"], f).T
    p["wpvT"] = wpv
    p["b_pv"] = c(
        np.concatenate(
            [np.asarray(inputs["b_p"], f).ravel(), np.asarray(inputs["b_v"], f).ravel()]
        ).reshape(4, 1)
    )
    return p


LAST_RESULT = None


def kernel(**inputs):
    global LAST_RESULT
    from concourse.bass_utils import run_bass_kernel_spmd

    if "nc" not in _CACHED:
        _CACHED["nc"] = _build()
    nc = _CACHED["nc"]

    f = np.float32
    img = np.asarray(inputs["img_feat"], f)
    hx = np.asarray(inputs["_hx"], f)
    cx = np.asarray(inputs["_cx"], f)
    q = np.asarray(inputs["query"], f)
    idx = np.asarray(inputs["instruction_idx"]).reshape(-1).astype(np.int64)

    params = _prep_params(inputs)

    in_maps = []
    for core in range(NCORES):
        rows = slice(core * R, (core + 1) * R)
        oh = (idx[rows][None, :] == np.arange(4)[:, None]).astype(f)
        m = {
            "imgT": np.ascontiguousarray(img[rows].T),
            "hxT": np.ascontiguousarray(hx[rows].T),
            "cxT": np.ascontiguousarray(cx[rows].T),
            "queryT": np.ascontiguousarray(q[rows].T),
            "ohT": np.ascontiguousarray(oh),
        }
        m.update(params)
        in_maps.append(m)

    res = run_bass_kernel_spmd(nc, in_maps, core_ids=list(range(NCORES)))
    LAST_RESULT = res

    hx_out = np.empty((B, 256), f)
    cx_out = np.empty((B, 256), f)
    val_out = np.empty((B, 1), f)
    pol_out = np.empty((B, 3), f)
    for core in range(NCORES):
        rows = slice(core * R, (core + 1) * R)
        r = res.results[core]
        hx_out[rows] = r["hxT_out"].T
        cx_out[rows] = r["cxT_out"].T
        pol_out[rows] = r["pvT_out"][0:3].T
        val_out[rows] = r["pvT_out"][3:4].T
    return (val_out, pol_out, hx_out, cx_out)
